# revision 13
# baseline (speedup 1.0000x reference)
"""Circular correlation 1D (FFT reference) as a direct 9-tap conv on TRN2.

Math: the reference's ortho-normalized FFT round trip reduces exactly to
    y[b, o, m] = sum_i sum_t K[o, i, t] * x[b, i, (m + t) mod N] + bias[o]
so we compute it as 9 PSUM-accumulated matmuls per output chunk:
    lhsT = K[:, :, t]^T  (shape [c_in=128, c_out=128], c_in on partitions)
    rhs  = x[b, :, m0+t : m0+t+CHUNK]  (c_in on partitions)
Sharding: pure data-parallel over batch — 32 batches / 8 cores = 4 each.
Each core computes its full [c_out=128, N=4096] slab; no collectives.
"""

import sys

if "/opt/trn_rl_repo" not in sys.path:
    sys.path.insert(0, "/opt/trn_rl_repo")

import numpy as np

import concourse.bass as bass
import concourse.mybir as mybir
import concourse.tile as tile
from concourse import bacc
from concourse.bass_utils import run_bass_kernel_spmd

B, C_IN, C_OUT, KS, N = 32, 128, 128, 9, 4096
N_CORES = 8
BPC = B // N_CORES  # batches per core
CHUNK = 512  # one PSUM bank of fp32
N_CHUNKS = N // CHUNK
HALO = KS - 1

_DT_F32 = mybir.dt.float32
_DT_F32R = mybir.dt.float32r  # full-rate fp32 matmul mode (free dim >= 256)


def build_nc() -> bass.Bass:
    nc = bacc.Bacc()
    x_ext = nc.dram_tensor("x", [BPC, C_IN, N + HALO], _DT_F32R, kind="ExternalInput")
    w_ext = nc.dram_tensor("w", [C_IN, KS * C_OUT], _DT_F32R, kind="ExternalInput")
    b_ext = nc.dram_tensor("b", [C_OUT, 1], _DT_F32, kind="ExternalInput")
    y_ext = nc.dram_tensor("y", [BPC, C_OUT, N], _DT_F32, kind="ExternalOutput")

    with tile.TileContext(nc) as tc:
        with (
            tc.tile_pool(name="const", bufs=1) as cpool,
            # one slot per batch — x slots never reused, so x-in DMAs carry no
            # WAR wait (leaves their single wait slot for ring flow control)
            tc.tile_pool(name="xin", bufs=BPC) as xpool,
            tc.tile_pool(name="psum", bufs=4, space="PSUM") as ppool,
            # Half-batch output staging; one slot per half-batch over the whole
            # kernel (never reused) so no instruction ever carries a WAR wait —
            # every engine instruction has a single sync-wait slot. Staging
            # also keeps the total DMA count low enough (<4 per HWDGE queue)
            # that Tile never adds ring flow-control waits to DMA triggers.
            tc.tile_pool(name="out", bufs=2 * BPC) as opool,
        ):
            w_t = cpool.tile([C_IN, KS * C_OUT], _DT_F32R)
            nc.sync.dma_start(out=w_t[:], in_=w_ext[:])
            bias_t = cpool.tile([C_OUT, 1], _DT_F32)
            nc.sync.dma_start(out=bias_t[:], in_=b_ext[:])

            # Engine instructions have a single sync-wait slot. Absorb each
            # DMA-completion wait into a dummy op on the engine that will
            # consume the data, so no compute instruction ever needs >1 wait:
            # dummy LDWEIGHTS (PE) for w/x, dummy activation (ACT) for bias.
            nc.tensor.ldweights(w_t[:].bitcast(mybir.dt.bfloat16)[:, 0:C_OUT])
            bias_warm = cpool.tile([C_OUT, 1], _DT_F32)
            nc.scalar.activation(
                bias_warm[:], bias_t[:], mybir.ActivationFunctionType.Identity
            )

            for b in range(BPC):
                x_t = xpool.tile([C_IN, N + HALO], _DT_F32R)
                nc.sync.dma_start(out=x_t[:], in_=x_ext[b])
                nc.tensor.ldweights(x_t[:].bitcast(mybir.dt.bfloat16)[:, 0:C_OUT])
                for h in range(2):
                    half = N // 2
                    stage = opool.tile([C_OUT, half], _DT_F32)
                    for cc in range(N_CHUNKS // 2):
                        m0 = h * half + cc * CHUNK
                        ps = ppool.tile([C_OUT, CHUNK], _DT_F32)
                        for t in range(KS):
                            nc.tensor.matmul(
                                ps[:],
                                w_t[:, t * C_OUT : (t + 1) * C_OUT],
                                x_t[:, m0 + t : m0 + t + CHUNK],
                                start=(t == 0),
                                stop=(t == KS - 1),
                            )
                        nc.scalar.activation(
                            stage[:, cc * CHUNK : (cc + 1) * CHUNK],
                            ps[:],
                            mybir.ActivationFunctionType.Identity,
                            bias=bias_t[:],
                        )
                    # SWDGE (gpsimd) path: ring flow control lives in the Q7
                    # software loop, not in trigger-instruction semaphores, so
                    # this DMA's single wait slot is free for the ACT RAW dep.
                    nc.gpsimd.dma_start(
                        out=y_ext[b, :, h * half : (h + 1) * half], in_=stage[:]
                    )
    # Legalize: splits any instruction with >1 sync wait into EventSemaphore
    # chains (TRN2 allows one wait per instruction), register alloc, DCE.
    nc.compile()
    return nc


def _prep_inputs(x: np.ndarray, k: np.ndarray, bias: np.ndarray):
    # circular halo so every rhs slice is contiguous in SBUF
    x_pad = np.concatenate([x, x[:, :, :HALO]], axis=-1)
    # w[i, t*C_OUT + o] = k[o, i, t]  -> lhsT slice [:, t*128:(t+1)*128] is [i, o]
    w = np.ascontiguousarray(k.transpose(1, 2, 0)).reshape(C_IN, KS * C_OUT)
    b2 = np.ascontiguousarray(bias.reshape(C_OUT, 1))
    in_maps = [
        {
            "x": np.ascontiguousarray(x_pad[c * BPC : (c + 1) * BPC]),
            "w": w,
            "b": b2,
        }
        for c in range(N_CORES)
    ]
    return in_maps


_NC_CACHE = []


def kernel(**inputs: np.ndarray) -> np.ndarray:
    x = np.asarray(inputs["x"], dtype=np.float32)
    k = np.asarray(inputs["kernel"], dtype=np.float32)
    bias = np.asarray(inputs["bias"], dtype=np.float32)
    assert x.shape == (B, C_IN, N) and k.shape == (C_OUT, C_IN, KS)

    if not _NC_CACHE:
        _NC_CACHE.append(build_nc())
    nc = _NC_CACHE[0]

    in_maps = _prep_inputs(x, k, bias)
    res = run_bass_kernel_spmd(nc, in_maps, list(range(N_CORES)))
    y = np.concatenate([res.results[c]["y"] for c in range(N_CORES)], axis=0)
    return y.astype(np.float32, copy=False)


# revision 14
# speedup vs baseline: 1.3306x; 1.3306x over previous
"""Circular correlation 1D (FFT reference) as a direct 9-tap conv on TRN2.

Math: the reference's ortho-normalized FFT round trip reduces exactly to
    y[b, o, m] = sum_i sum_t K[o, i, t] * x[b, i, (m + t) mod N] + bias[o]
so we compute it as 9 PSUM-accumulated float32r matmuls per 512-col chunk:
    lhsT = K[:, :, t]^T  (shape [c_in=128, c_out=128], c_in on partitions)
    rhs  = x[b, :, m0+t : m0+t+512]    (c_in on partitions)
float32r runs the PE at full rate (1 cycle/row) for free dims >= 256, vs 4
cycles/row for plain fp32 — per-core PE floor is ~61.4 us for this shape.

Sharding: pure data-parallel over batch — 32 batches / 8 cores = 4 each.
Each core computes its full [c_out=128, N=4096] slab; no collectives.

Pipeline notes (all verified against the instruction cost-model timeline):
- x arrives with an 8-col circular halo (host-padded) so every matmul rhs is
  a contiguous SBUF slice.
- batch 0's x lands in 3 sub-DMAs (1024/1540/1540 cols) so the first matmul
  starts as soon as the w DMA + the first piece land (~6 us) instead of
  waiting for the whole 2 MB batch (~10.5 us).
- outputs stage in quarter-batch SBUF tiles (never-reused slots) and the
  final quarter is split in two so the last DMA is short.
- dummy LDWEIGHTS/activation ops absorb DMA-completion waits onto the
  consuming engine; remaining multi-wait instructions are legalized by
  Bacc.compile()'s generate_event_semaphores pass (TRN2 allows one sync
  wait per engine instruction).
"""

import sys

if "/opt/trn_rl_repo" not in sys.path:
    sys.path.insert(0, "/opt/trn_rl_repo")

import numpy as np

import concourse.bass as bass
import concourse.mybir as mybir
import concourse.tile as tile
from concourse import bacc
from concourse.bass_utils import run_bass_kernel_spmd

B, C_IN, C_OUT, KS, N = 32, 128, 128, 9, 4096
N_CORES = 8
BPC = B // N_CORES  # batches per core
CHUNK = 512  # one PSUM bank of fp32; max fp32 moving free-dim
N_CHUNKS = N // CHUNK
HALO = KS - 1
OUT_PARTS = 4  # quarter-batch output staging
TAIL_SPLIT = 2  # split the very last quarter's DMA in two

_DT_F32 = mybir.dt.float32
_DT_F32R = mybir.dt.float32r  # full-rate fp32 matmul mode (free dim >= 256)


def build_nc() -> bass.Bass:
    nc = bacc.Bacc()
    x_ext = nc.dram_tensor("x", [BPC, C_IN, N + HALO], _DT_F32R, kind="ExternalInput")
    w_ext = nc.dram_tensor("w", [C_IN, KS * C_OUT], _DT_F32R, kind="ExternalInput")
    b_ext = nc.dram_tensor("b", [C_OUT, 1], _DT_F32, kind="ExternalInput")
    y_ext = nc.dram_tensor("y", [BPC, C_OUT, N], _DT_F32, kind="ExternalOutput")

    with tile.TileContext(nc) as tc:
        with (
            tc.tile_pool(name="const", bufs=1) as cpool,
            # one slot per batch — x slots are never reused, so x-in DMAs
            # carry no WAR wait
            tc.tile_pool(name="xin", bufs=BPC) as xpool,
            tc.tile_pool(name="psum", bufs=8, space="PSUM") as ppool,
            # never-reused staging slots: ACT writes carry no WAR waits
            tc.tile_pool(name="out", bufs=OUT_PARTS * BPC - 1) as opool,
            tc.tile_pool(name="tail", bufs=TAIL_SPLIT) as tpool,
        ):
            w_t = cpool.tile([C_IN, KS * C_OUT], _DT_F32R)
            nc.sync.dma_start(out=w_t[:], in_=w_ext[:])
            bias_t = cpool.tile([C_OUT, 1], _DT_F32)
            nc.sync.dma_start(out=bias_t[:], in_=b_ext[:])

            # Absorb DMA-completion waits into dummy ops on the consuming
            # engine so hot-path instructions keep a single sync wait.
            nc.tensor.ldweights(w_t[:].bitcast(mybir.dt.bfloat16)[:, 0:C_OUT])
            bias_warm = cpool.tile([C_OUT, 1], _DT_F32)
            nc.scalar.activation(
                bias_warm[:], bias_t[:], mybir.ActivationFunctionType.Identity
            )

            part = N // OUT_PARTS
            for b in range(BPC):
                x_t = xpool.tile([C_IN, N + HALO], _DT_F32R)
                if b == 0:
                    # lead with a small piece so compute starts early
                    cuts = [0, 1024, 1024 + (N + HALO - 1024) // 2, N + HALO]
                else:
                    cuts = [0, (N + HALO) // 2, N + HALO]
                for s, e in zip(cuts[:-1], cuts[1:]):
                    nc.sync.dma_start(out=x_t[:, s:e], in_=x_ext[b, :, s:e])
                nc.tensor.ldweights(x_t[:].bitcast(mybir.dt.bfloat16)[:, 0:C_OUT])

                for h in range(OUT_PARTS):
                    last_part = b == BPC - 1 and h == OUT_PARTS - 1
                    nsub = TAIL_SPLIT if last_part else 1
                    sub = part // nsub
                    for u in range(nsub):
                        pool_ = tpool if last_part else opool
                        stage = pool_.tile(
                            [C_OUT, sub],
                            _DT_F32,
                            tag="tailst" if last_part else "stage",
                        )
                        for cc in range(sub // CHUNK):
                            m0 = h * part + u * sub + cc * CHUNK
                            ps = ppool.tile([C_OUT, CHUNK], _DT_F32)
                            for t in range(KS):
                                nc.tensor.matmul(
                                    ps[:],
                                    w_t[:, t * C_OUT : (t + 1) * C_OUT],
                                    x_t[:, m0 + t : m0 + t + CHUNK],
                                    start=(t == 0),
                                    stop=(t == KS - 1),
                                )
                            nc.scalar.activation(
                                stage[:, cc * CHUNK : (cc + 1) * CHUNK],
                                ps[:],
                                mybir.ActivationFunctionType.Identity,
                                bias=bias_t[:],
                            )
                        nc.sync.dma_start(
                            out=y_ext[b, :, h * part + u * sub : h * part + (u + 1) * sub],
                            in_=stage[:],
                        )
    # Legalize: splits any instruction with >1 sync wait into EventSemaphore
    # chains (TRN2 allows one wait per instruction), register alloc, DCE.
    nc.compile()
    return nc


def _prep_inputs(x: np.ndarray, k: np.ndarray, bias: np.ndarray):
    # circular halo so every rhs slice is contiguous in SBUF
    x_pad = np.concatenate([x, x[:, :, :HALO]], axis=-1)
    # w[i, t*C_OUT + o] = k[o, i, t]  -> lhsT slice [:, t*128:(t+1)*128] is [i, o]
    w = np.ascontiguousarray(k.transpose(1, 2, 0)).reshape(C_IN, KS * C_OUT)
    b2 = np.ascontiguousarray(bias.reshape(C_OUT, 1))
    in_maps = [
        {
            "x": np.ascontiguousarray(x_pad[c * BPC : (c + 1) * BPC]),
            "w": w,
            "b": b2,
        }
        for c in range(N_CORES)
    ]
    return in_maps


_NC_CACHE = []


def kernel(**inputs: np.ndarray) -> np.ndarray:
    x = np.asarray(inputs["x"], dtype=np.float32)
    k = np.asarray(inputs["kernel"], dtype=np.float32)
    bias = np.asarray(inputs["bias"], dtype=np.float32)
    assert x.shape == (B, C_IN, N) and k.shape == (C_OUT, C_IN, KS)

    if not _NC_CACHE:
        _NC_CACHE.append(build_nc())
    nc = _NC_CACHE[0]

    in_maps = _prep_inputs(x, k, bias)
    res = run_bass_kernel_spmd(nc, in_maps, list(range(N_CORES)))
    y = np.concatenate([res.results[c]["y"] for c in range(N_CORES)], axis=0)
    return y.astype(np.float32, copy=False)


# revision 16
# speedup vs baseline: 1.3451x; 1.0109x over previous
"""Circular correlation 1D (FFT reference) as a direct 9-tap conv on TRN2.

Math: the reference's ortho-normalized FFT round trip reduces exactly to
    y[b, o, m] = sum_i sum_t K[o, i, t] * x[b, i, (m + t) mod N] + bias[o]
so we compute it as 9 PSUM-accumulated float32r matmuls per 512-col chunk:
    lhsT = K[:, :, t]^T  (shape [c_in=128, c_out=128], c_in on partitions)
    rhs  = x[b, :, m0+t : m0+t+512]    (c_in on partitions)
float32r runs the PE at full rate (1 cycle/row) for free dims >= 256, vs 4
cycles/row for plain fp32 — per-core PE floor is ~61.4 us for this shape.

Sharding: pure data-parallel over batch — 32 batches / 8 cores = 4 each.
Each core computes its full [c_out=128, N=4096] slab; no collectives.

Pipeline notes (all verified against the instruction cost-model timeline):
- x arrives with an 8-col circular halo (host-padded) so every matmul rhs is
  a contiguous SBUF slice.
- batch 0's x lands in 3 sub-DMAs (1024/1540/1540 cols) so the first matmul
  starts as soon as the w DMA + the first piece land (~6 us) instead of
  waiting for the whole 2 MB batch (~10.5 us).
- outputs stage in quarter-batch SBUF tiles (never-reused slots) and the
  final quarter is split in two so the last DMA is short.
- dummy LDWEIGHTS/activation ops absorb DMA-completion waits onto the
  consuming engine; remaining multi-wait instructions are legalized by
  Bacc.compile()'s generate_event_semaphores pass (TRN2 allows one sync
  wait per engine instruction).
"""

import sys

if "/opt/trn_rl_repo" not in sys.path:
    sys.path.insert(0, "/opt/trn_rl_repo")

import numpy as np

import concourse.bass as bass
import concourse.mybir as mybir
import concourse.tile as tile
from concourse import bacc
from concourse.bass_utils import run_bass_kernel_spmd

B, C_IN, C_OUT, KS, N = 32, 128, 128, 9, 4096
N_CORES = 8
BPC = B // N_CORES  # batches per core
CHUNK = 512  # one PSUM bank of fp32; max fp32 moving free-dim
N_CHUNKS = N // CHUNK
HALO = KS - 1
OUT_PARTS = 4  # quarter-batch output staging
TAIL_SPLIT = 2  # split the very last quarter's DMA in two

_DT_F32 = mybir.dt.float32
_DT_F32R = mybir.dt.float32r  # full-rate fp32 matmul mode (free dim >= 256)


def build_nc() -> bass.Bass:
    nc = bacc.Bacc()
    x_ext = nc.dram_tensor("x", [BPC, C_IN, N + HALO], _DT_F32R, kind="ExternalInput")
    w_ext = nc.dram_tensor("w", [C_IN, KS * C_OUT], _DT_F32R, kind="ExternalInput")
    b_ext = nc.dram_tensor("b", [C_OUT, 1], _DT_F32, kind="ExternalInput")
    y_ext = nc.dram_tensor("y", [BPC, C_OUT, N], _DT_F32, kind="ExternalOutput")

    with tile.TileContext(nc) as tc:
        with (
            tc.tile_pool(name="const", bufs=1) as cpool,
            # one slot per batch — x slots are never reused, so x-in DMAs
            # carry no WAR wait
            tc.tile_pool(name="xin", bufs=BPC) as xpool,
            tc.tile_pool(name="psum", bufs=8, space="PSUM") as ppool,
            # never-reused staging slots: ACT writes carry no WAR waits
            tc.tile_pool(name="out", bufs=OUT_PARTS * BPC - 1) as opool,
            tc.tile_pool(name="tail", bufs=TAIL_SPLIT) as tpool,
        ):
            # w/bias ride the SWDGE (gpsimd) ring so the sync HWDGE ring is
            # free to start streaming x immediately.
            w_t = cpool.tile([C_IN, KS * C_OUT], _DT_F32R)
            nc.gpsimd.dma_start(out=w_t[:], in_=w_ext[:])
            bias_t = cpool.tile([C_OUT, 1], _DT_F32)
            nc.gpsimd.dma_start(out=bias_t[:], in_=b_ext[:])

            # Absorb DMA-completion waits into dummy ops on the consuming
            # engine so hot-path instructions keep a single sync wait.
            nc.tensor.ldweights(w_t[:].bitcast(mybir.dt.bfloat16)[:, 0:C_OUT])
            bias_warm = cpool.tile([C_OUT, 1], _DT_F32)
            nc.scalar.activation(
                bias_warm[:], bias_t[:], mybir.ActivationFunctionType.Identity
            )

            part = N // OUT_PARTS
            for b in range(BPC):
                x_t = xpool.tile([C_IN, N + HALO], _DT_F32R)
                if b == 0:
                    # lead with a small piece so compute starts early (must
                    # cover chunk 0's reads: >= CHUNK + HALO = 520 cols)
                    cuts = [0, 640, 640 + (N + HALO - 640) // 2, N + HALO]
                else:
                    cuts = [0, (N + HALO) // 2, N + HALO]
                for s, e in zip(cuts[:-1], cuts[1:]):
                    nc.sync.dma_start(out=x_t[:, s:e], in_=x_ext[b, :, s:e])
                nc.tensor.ldweights(x_t[:].bitcast(mybir.dt.bfloat16)[:, 0:C_OUT])

                for h in range(OUT_PARTS):
                    last_part = b == BPC - 1 and h == OUT_PARTS - 1
                    nsub = TAIL_SPLIT if last_part else 1
                    sub = part // nsub
                    for u in range(nsub):
                        pool_ = tpool if last_part else opool
                        stage = pool_.tile(
                            [C_OUT, sub],
                            _DT_F32,
                            tag="tailst" if last_part else "stage",
                        )
                        for cc in range(sub // CHUNK):
                            m0 = h * part + u * sub + cc * CHUNK
                            ps = ppool.tile([C_OUT, CHUNK], _DT_F32)
                            for t in range(KS):
                                nc.tensor.matmul(
                                    ps[:],
                                    w_t[:, t * C_OUT : (t + 1) * C_OUT],
                                    x_t[:, m0 + t : m0 + t + CHUNK],
                                    start=(t == 0),
                                    stop=(t == KS - 1),
                                )
                            nc.scalar.activation(
                                stage[:, cc * CHUNK : (cc + 1) * CHUNK],
                                ps[:],
                                mybir.ActivationFunctionType.Identity,
                                bias=bias_t[:],
                            )
                        nc.sync.dma_start(
                            out=y_ext[b, :, h * part + u * sub : h * part + (u + 1) * sub],
                            in_=stage[:],
                        )
    # Legalize: splits any instruction with >1 sync wait into EventSemaphore
    # chains (TRN2 allows one wait per instruction), register alloc, DCE.
    nc.compile()
    return nc


def _prep_inputs(x: np.ndarray, k: np.ndarray, bias: np.ndarray):
    # circular halo so every rhs slice is contiguous in SBUF
    x_pad = np.concatenate([x, x[:, :, :HALO]], axis=-1)
    # w[i, t*C_OUT + o] = k[o, i, t]  -> lhsT slice [:, t*128:(t+1)*128] is [i, o]
    w = np.ascontiguousarray(k.transpose(1, 2, 0)).reshape(C_IN, KS * C_OUT)
    b2 = np.ascontiguousarray(bias.reshape(C_OUT, 1))
    in_maps = [
        {
            "x": np.ascontiguousarray(x_pad[c * BPC : (c + 1) * BPC]),
            "w": w,
            "b": b2,
        }
        for c in range(N_CORES)
    ]
    return in_maps


_NC_CACHE = []


def kernel(**inputs: np.ndarray) -> np.ndarray:
    x = np.asarray(inputs["x"], dtype=np.float32)
    k = np.asarray(inputs["kernel"], dtype=np.float32)
    bias = np.asarray(inputs["bias"], dtype=np.float32)
    assert x.shape == (B, C_IN, N) and k.shape == (C_OUT, C_IN, KS)

    if not _NC_CACHE:
        _NC_CACHE.append(build_nc())
    nc = _NC_CACHE[0]

    in_maps = _prep_inputs(x, k, bias)
    res = run_bass_kernel_spmd(nc, in_maps, list(range(N_CORES)))
    y = np.concatenate([res.results[c]["y"] for c in range(N_CORES)], axis=0)
    return y.astype(np.float32, copy=False)


# revision 17
# speedup vs baseline: 1.4320x; 1.0645x over previous
"""Circular correlation 1D (FFT reference) as a direct 9-tap conv on TRN2.

Math: the reference's ortho-normalized FFT round trip reduces exactly to
    y[b, o, m] = sum_i sum_t K[o, i, t] * x[b, i, (m + t) mod N] + bias[o]
so we compute it as 9 PSUM-accumulated float32r matmuls per 512-col chunk:
    lhsT = K[:, :, t]^T  (shape [c_in=128, c_out=128], c_in on partitions)
    rhs  = x[b, :, m0+t : m0+t+512]    (c_in on partitions)
float32r runs the PE at full rate (1 cycle/row) for free dims >= 256, vs 4
cycles/row for plain fp32 — per-core PE floor is ~61.4 us for this shape.

Sharding: pure data-parallel over batch — 32 batches / 8 cores = 4 each.
Each core computes its full [c_out=128, N=4096] slab; no collectives.

Pipeline structure (tuned against the instruction cost-model timeline; the
PE runs gap-free from the first to the last matmul):
- head: DMAs are emitted in critical-path order — w taps 0-2, then x batch 0
  cols 0-519 (exactly chunk 0's reads), then w taps 3-8, then the rest of
  x batch 0 in 512/512/remainder pieces. First matmul issues ~4.3 us in,
  with tap t and chunk c data always landing just ahead of use.
- x arrives with an 8-col circular halo (host-padded) so every matmul rhs is
  a contiguous SBUF slice; batches 1-3 stream in halves during compute.
- outputs stage in quarter-batch SBUF tiles (never-reused slots); the final
  quarter is written out as 512+256+256 cols so the last DMA is short.
- dummy bf16 LDWEIGHTS absorb DMA-completion waits onto the PE; remaining
  multi-wait instructions are legalized by Bacc.compile()'s
  generate_event_semaphores pass (TRN2 allows one sync wait per engine
  instruction — building with plain bass.Bass() fails walrus codegen).
"""

import sys

if "/opt/trn_rl_repo" not in sys.path:
    sys.path.insert(0, "/opt/trn_rl_repo")

import numpy as np

import concourse.bass as bass
import concourse.mybir as mybir
import concourse.tile as tile
from concourse import bacc
from concourse.bass_utils import run_bass_kernel_spmd

B, C_IN, C_OUT, KS, N = 32, 128, 128, 9, 4096
N_CORES = 8
BPC = B // N_CORES  # batches per core
CHUNK = 512  # one PSUM bank of fp32; max fp32 moving free-dim
HALO = KS - 1
OUT_PARTS = 4  # quarter-batch output staging
W_SPLIT = (3, 6)  # w DMA pieces (taps): first piece unblocks chunk 0
X0_PIECES = (520, 512, 512)  # leading x-batch-0 pieces (remainder appended)
TAIL_SUBS = (512, 256, 256)  # final quarter written in shrinking pieces

_DT_F32 = mybir.dt.float32
_DT_F32R = mybir.dt.float32r  # full-rate fp32 matmul mode (free dim >= 256)


def build_nc() -> bass.Bass:
    nc = bacc.Bacc()
    x_ext = nc.dram_tensor("x", [BPC, C_IN, N + HALO], _DT_F32R, kind="ExternalInput")
    w_ext = nc.dram_tensor("w", [C_IN, KS * C_OUT], _DT_F32R, kind="ExternalInput")
    b_ext = nc.dram_tensor("b", [C_OUT, 1], _DT_F32, kind="ExternalInput")
    y_ext = nc.dram_tensor("y", [BPC, C_OUT, N], _DT_F32, kind="ExternalOutput")

    with tile.TileContext(nc) as tc:
        with (
            tc.tile_pool(name="const", bufs=1) as cpool,
            # distinct tag per batch: x slots never reused -> no WAR waits
            tc.tile_pool(name="xin", bufs=1) as xpool,
            tc.tile_pool(name="psum", bufs=8, space="PSUM") as ppool,
            # never-reused staging slots: ACT writes carry no WAR waits
            tc.tile_pool(name="out", bufs=OUT_PARTS * BPC - 1) as opool,
            tc.tile_pool(name="tail", bufs=1) as tpool,
        ):
            w_t = cpool.tile([C_IN, KS * C_OUT], _DT_F32R)
            bias_t = cpool.tile([C_OUT, 1], _DT_F32)
            x_tiles = []
            for b in range(BPC):
                xt = xpool.tile([C_IN, N + HALO], _DT_F32R, tag=f"x{b}")
                x_tiles.append(xt)
            wbf = w_t[:].bitcast(mybir.dt.bfloat16)

            def w_piece(t0, npiece):
                sl = slice(t0 * C_OUT, (t0 + npiece) * C_OUT)
                nc.sync.dma_start(out=w_t[:, sl], in_=w_ext[:, sl])
                # dummy bf16 LDWEIGHTS inside this piece: absorbs the DMA
                # wait on the PE queue (fp32r matmuls self-load weights, so
                # the loaded garbage is never used)
                nc.tensor.ldweights(wbf[:, 2 * t0 * C_OUT : 2 * t0 * C_OUT + C_OUT])

            def x_piece(b, s, e):
                nc.sync.dma_start(out=x_tiles[b][:, s:e], in_=x_ext[b, :, s:e])
                xbf = x_tiles[b][:].bitcast(mybir.dt.bfloat16)
                nc.tensor.ldweights(xbf[:, 2 * s : 2 * s + C_OUT])

            # critical-path-ordered head: w piece 1, x0 piece 1 (chunk 0's
            # data), w piece 2, bias, then the rest of x batch 0
            t0 = 0
            w_piece(t0, W_SPLIT[0])
            t0 += W_SPLIT[0]
            cuts0 = [0]
            for p in X0_PIECES:
                cuts0.append(cuts0[-1] + p)
            cuts0.append(N + HALO)
            x_piece(0, cuts0[0], cuts0[1])
            for npiece in W_SPLIT[1:]:
                w_piece(t0, npiece)
                t0 += npiece
            nc.sync.dma_start(out=bias_t[:], in_=b_ext[:])
            bias_warm = cpool.tile([C_OUT, 1], _DT_F32)
            nc.scalar.activation(
                bias_warm[:], bias_t[:], mybir.ActivationFunctionType.Identity
            )
            for s, e in zip(cuts0[1:-1], cuts0[2:]):
                if e > s:
                    x_piece(0, s, e)
            for b in range(1, BPC):
                half = (N + HALO + 1) // 2
                for s, e in ((0, half), (half, N + HALO)):
                    x_piece(b, s, e)

            part = N // OUT_PARTS
            for b in range(BPC):
                x_t = x_tiles[b]
                for h in range(OUT_PARTS):
                    last_part = b == BPC - 1 and h == OUT_PARTS - 1
                    subs = list(TAIL_SUBS) if last_part else [part]
                    off = 0
                    for ui, sub in enumerate(subs):
                        pool_ = tpool if last_part else opool
                        stage = pool_.tile(
                            [C_OUT, sub],
                            _DT_F32,
                            tag=f"tail{ui}" if last_part else "stage",
                        )
                        for cc in range(max(1, sub // CHUNK)):
                            w_cols = min(sub, CHUNK)
                            m0 = h * part + off + cc * w_cols
                            ps = ppool.tile([C_OUT, w_cols], _DT_F32, tag="ps")
                            for t in range(KS):
                                nc.tensor.matmul(
                                    ps[:],
                                    w_t[:, t * C_OUT : (t + 1) * C_OUT],
                                    x_t[:, m0 + t : m0 + t + w_cols],
                                    start=(t == 0),
                                    stop=(t == KS - 1),
                                )
                            nc.scalar.activation(
                                stage[:, cc * w_cols : (cc + 1) * w_cols],
                                ps[:],
                                mybir.ActivationFunctionType.Identity,
                                bias=bias_t[:],
                            )
                        nc.sync.dma_start(
                            out=y_ext[b, :, h * part + off : h * part + off + sub],
                            in_=stage[:],
                        )
                        off += sub
    # Legalize: splits any instruction with >1 sync wait into EventSemaphore
    # chains (TRN2 allows one wait per instruction), register alloc, DCE.
    nc.compile()
    return nc


def _prep_inputs(x: np.ndarray, k: np.ndarray, bias: np.ndarray):
    # circular halo so every rhs slice is contiguous in SBUF
    x_pad = np.concatenate([x, x[:, :, :HALO]], axis=-1)
    # w[i, t*C_OUT + o] = k[o, i, t]  -> lhsT slice [:, t*128:(t+1)*128] is [i, o]
    w = np.ascontiguousarray(k.transpose(1, 2, 0)).reshape(C_IN, KS * C_OUT)
    b2 = np.ascontiguousarray(bias.reshape(C_OUT, 1))
    in_maps = [
        {
            "x": np.ascontiguousarray(x_pad[c * BPC : (c + 1) * BPC]),
            "w": w,
            "b": b2,
        }
        for c in range(N_CORES)
    ]
    return in_maps


_NC_CACHE = []


def kernel(**inputs: np.ndarray) -> np.ndarray:
    x = np.asarray(inputs["x"], dtype=np.float32)
    k = np.asarray(inputs["kernel"], dtype=np.float32)
    bias = np.asarray(inputs["bias"], dtype=np.float32)
    assert x.shape == (B, C_IN, N) and k.shape == (C_OUT, C_IN, KS)

    if not _NC_CACHE:
        _NC_CACHE.append(build_nc())
    nc = _NC_CACHE[0]

    in_maps = _prep_inputs(x, k, bias)
    res = run_bass_kernel_spmd(nc, in_maps, list(range(N_CORES)))
    y = np.concatenate([res.results[c]["y"] for c in range(N_CORES)], axis=0)
    return y.astype(np.float32, copy=False)


# revision 19
# speedup vs baseline: 1.4333x; 1.0009x over previous
"""Circular correlation 1D (FFT reference) as a direct 9-tap conv on TRN2.

Math: the reference's ortho-normalized FFT round trip reduces exactly to
    y[b, o, m] = sum_i sum_t K[o, i, t] * x[b, i, (m + t) mod N] + bias[o]
so we compute it as 9 PSUM-accumulated float32r matmuls per 512-col chunk:
    lhsT = K[:, :, t]^T  (shape [c_in=128, c_out=128], c_in on partitions)
    rhs  = x[b, :, m0+t : m0+t+512]    (c_in on partitions)
float32r runs the PE at full rate (1 cycle/row) for free dims >= 256, vs 4
cycles/row for plain fp32 — per-core PE floor is ~61.4 us for this shape.

Sharding: pure data-parallel over batch — 32 batches / 8 cores = 4 each.
Each core computes its full [c_out=128, N=4096] slab; no collectives.

Pipeline structure (tuned against the instruction cost-model timeline; the
PE runs gap-free from the first to the last matmul):
- head: DMAs are emitted in critical-path order — w taps 0-2, then x batch 0
  cols 0-519 (exactly chunk 0's reads), then w taps 3-8, then the rest of
  x batch 0 in 512/512/remainder pieces. First matmul issues ~4.3 us in,
  with tap t and chunk c data always landing just ahead of use.
- x arrives with an 8-col circular halo (host-padded) so every matmul rhs is
  a contiguous SBUF slice; batches 1-3 stream in halves during compute.
- outputs stage in quarter-batch SBUF tiles (never-reused slots); the final
  quarter is written out as 512+256+256 cols so the last DMA is short.
- dummy bf16 LDWEIGHTS absorb DMA-completion waits onto the PE; remaining
  multi-wait instructions are legalized by Bacc.compile()'s
  generate_event_semaphores pass (TRN2 allows one sync wait per engine
  instruction — building with plain bass.Bass() fails walrus codegen).
"""

import sys

if "/opt/trn_rl_repo" not in sys.path:
    sys.path.insert(0, "/opt/trn_rl_repo")

import numpy as np

import concourse.bass as bass
import concourse.mybir as mybir
import concourse.tile as tile
from concourse import bacc
from concourse.bass_utils import run_bass_kernel_spmd

B, C_IN, C_OUT, KS, N = 32, 128, 128, 9, 4096
N_CORES = 8
BPC = B // N_CORES  # batches per core
CHUNK = 512  # one PSUM bank of fp32; max fp32 moving free-dim
HALO = KS - 1
OUT_PARTS = 4  # quarter-batch output staging
W_SPLIT = (3, 6)  # w DMA pieces (taps): first piece unblocks chunk 0
X0_PIECES = (520, 512, 512)  # leading x-batch-0 pieces (remainder appended)
TAIL_SUBS = (512, 256, 256)  # final quarter written in shrinking pieces

_DT_F32 = mybir.dt.float32
_DT_F32R = mybir.dt.float32r  # full-rate fp32 matmul mode (free dim >= 256)


def build_nc() -> bass.Bass:
    nc = bacc.Bacc()
    x_ext = nc.dram_tensor("x", [BPC, C_IN, N + HALO], _DT_F32R, kind="ExternalInput")
    w_ext = nc.dram_tensor("w", [C_IN, KS * C_OUT], _DT_F32R, kind="ExternalInput")
    b_ext = nc.dram_tensor("b", [C_OUT, 1], _DT_F32, kind="ExternalInput")
    y_ext = nc.dram_tensor("y", [BPC, C_OUT, N], _DT_F32, kind="ExternalOutput")

    with tile.TileContext(nc) as tc:
        with (
            tc.tile_pool(name="const", bufs=1) as cpool,
            # distinct tag per batch: x slots never reused -> no WAR waits
            tc.tile_pool(name="xin", bufs=1) as xpool,
            tc.tile_pool(name="psum", bufs=8, space="PSUM") as ppool,
            # never-reused staging slots: ACT writes carry no WAR waits
            tc.tile_pool(name="out", bufs=OUT_PARTS * BPC - 1) as opool,
            tc.tile_pool(name="tail", bufs=1) as tpool,
        ):
            w_t = cpool.tile([C_IN, KS * C_OUT], _DT_F32R)
            bias_t = cpool.tile([C_OUT, 1], _DT_F32)
            x_tiles = []
            for b in range(BPC):
                xt = xpool.tile([C_IN, N + HALO], _DT_F32R, tag=f"x{b}")
                x_tiles.append(xt)
            wbf = w_t[:].bitcast(mybir.dt.bfloat16)

            def w_piece(t0, npiece):
                sl = slice(t0 * C_OUT, (t0 + npiece) * C_OUT)
                nc.sync.dma_start(out=w_t[:, sl], in_=w_ext[:, sl])
                # dummy bf16 LDWEIGHTS inside this piece: absorbs the DMA
                # wait on the PE queue (fp32r matmuls self-load weights, so
                # the loaded garbage is never used)
                nc.tensor.ldweights(wbf[:, 2 * t0 * C_OUT : 2 * t0 * C_OUT + C_OUT])

            def x_piece(b, s, e):
                nc.sync.dma_start(out=x_tiles[b][:, s:e], in_=x_ext[b, :, s:e])
                xbf = x_tiles[b][:].bitcast(mybir.dt.bfloat16)
                nc.tensor.ldweights(xbf[:, 2 * s : 2 * s + C_OUT])

            # critical-path-ordered head: w piece 1, x0 piece 1 (chunk 0's
            # data), w piece 2, bias, then the rest of x batch 0
            t0 = 0
            w_piece(t0, W_SPLIT[0])
            t0 += W_SPLIT[0]
            cuts0 = [0]
            for p in X0_PIECES:
                cuts0.append(cuts0[-1] + p)
            cuts0.append(N + HALO)
            x_piece(0, cuts0[0], cuts0[1])
            for npiece in W_SPLIT[1:]:
                w_piece(t0, npiece)
                t0 += npiece
            nc.sync.dma_start(out=bias_t[:], in_=b_ext[:])
            bias_warm = cpool.tile([C_OUT, 1], _DT_F32)
            nc.scalar.activation(
                bias_warm[:], bias_t[:], mybir.ActivationFunctionType.Identity
            )
            bias_warm2 = cpool.tile([C_OUT, 1], _DT_F32)
            nc.vector.tensor_scalar_add(bias_warm2[:], bias_t[:], 0.0)
            for s, e in zip(cuts0[1:-1], cuts0[2:]):
                if e > s:
                    x_piece(0, s, e)
            for b in range(1, BPC):
                half = (N + HALO + 1) // 2
                for s, e in ((0, half), (half, N + HALO)):
                    x_piece(b, s, e)

            part = N // OUT_PARTS
            for b in range(BPC):
                x_t = x_tiles[b]
                for h in range(OUT_PARTS):
                    last_part = b == BPC - 1 and h == OUT_PARTS - 1
                    subs = list(TAIL_SUBS) if last_part else [part]
                    off = 0
                    for ui, sub in enumerate(subs):
                        pool_ = tpool if last_part else opool
                        stage = pool_.tile(
                            [C_OUT, sub],
                            _DT_F32,
                            tag=f"tail{ui}" if last_part else "stage",
                        )
                        for cc in range(max(1, sub // CHUNK)):
                            w_cols = min(sub, CHUNK)
                            m0 = h * part + off + cc * w_cols
                            ps = ppool.tile([C_OUT, w_cols], _DT_F32, tag="ps")
                            for t in range(KS):
                                nc.tensor.matmul(
                                    ps[:],
                                    w_t[:, t * C_OUT : (t + 1) * C_OUT],
                                    x_t[:, m0 + t : m0 + t + w_cols],
                                    start=(t == 0),
                                    stop=(t == KS - 1),
                                )
                            osl = stage[:, cc * w_cols : (cc + 1) * w_cols]
                            if last_part and ui >= 1:
                                # final two pieces drain on the (idle) DVE so
                                # the tail chain starts the moment the last
                                # matmul stops; fp32 add is bit-identical to
                                # the ACT bias path
                                nc.vector.tensor_scalar_add(osl, ps[:], bias_t[:])
                            else:
                                nc.scalar.activation(
                                    osl,
                                    ps[:],
                                    mybir.ActivationFunctionType.Identity,
                                    bias=bias_t[:],
                                )
                        nc.sync.dma_start(
                            out=y_ext[b, :, h * part + off : h * part + off + sub],
                            in_=stage[:],
                        )
                        off += sub
    # Legalize: splits any instruction with >1 sync wait into EventSemaphore
    # chains (TRN2 allows one wait per instruction), register alloc, DCE.
    nc.compile()
    return nc


def _prep_inputs(x: np.ndarray, k: np.ndarray, bias: np.ndarray):
    # circular halo so every rhs slice is contiguous in SBUF
    x_pad = np.concatenate([x, x[:, :, :HALO]], axis=-1)
    # w[i, t*C_OUT + o] = k[o, i, t]  -> lhsT slice [:, t*128:(t+1)*128] is [i, o]
    w = np.ascontiguousarray(k.transpose(1, 2, 0)).reshape(C_IN, KS * C_OUT)
    b2 = np.ascontiguousarray(bias.reshape(C_OUT, 1))
    in_maps = [
        {
            "x": np.ascontiguousarray(x_pad[c * BPC : (c + 1) * BPC]),
            "w": w,
            "b": b2,
        }
        for c in range(N_CORES)
    ]
    return in_maps


_NC_CACHE = []


def kernel(**inputs: np.ndarray) -> np.ndarray:
    x = np.asarray(inputs["x"], dtype=np.float32)
    k = np.asarray(inputs["kernel"], dtype=np.float32)
    bias = np.asarray(inputs["bias"], dtype=np.float32)
    assert x.shape == (B, C_IN, N) and k.shape == (C_OUT, C_IN, KS)

    if not _NC_CACHE:
        _NC_CACHE.append(build_nc())
    nc = _NC_CACHE[0]

    in_maps = _prep_inputs(x, k, bias)
    res = run_bass_kernel_spmd(nc, in_maps, list(range(N_CORES)))
    y = np.concatenate([res.results[c]["y"] for c in range(N_CORES)], axis=0)
    return y.astype(np.float32, copy=False)
